# revision 3
# baseline (speedup 1.0000x reference)
"""Trainium2 Bass kernel for nn_Brep_Gcn (GCN message passing), v2.

Math:
    x  = relu(conv1d(feature) summed over channels)   # banded matmul
    S1 = A @ x ; h = relu(S1 W1 + b1) ; P = h W2 ; y = A @ P + b2

Distribution:
  - conv is computed REPLICATED on every core (cheaper than AllGather).
  - L1 SpMM: edges partitioned by dest core; dest windows of 128; source
    chunks of 25000 (int16 gather); per-window PSUM accumulation across all
    4 chunks (group-of-4-windows PSUM bank tiles); dense W1/W2 GEMMs run
    per-window as soon as the window's S1 is final.
  - L2 SpMM: edges partitioned by SOURCE core: gather P rows from the LOCAL
    p_sh (no AllGather), scatter into global dest windows, partial logits
    written to y_part[100000, 32]; terminal ReduceScatter(add) yields each
    core's logit shard; + b2.

All SpMM data is fp16 (exact one-hot iota compares, 1 cyc/row PE matmuls,
2x DVE); PSUM accumulation fp32.
"""

import sys
from dataclasses import dataclass

import numpy as np

sys.path.insert(0, "/opt/trn_rl_repo")

import concourse.bass as bass
import concourse.tile as tile
from concourse import bacc
from concourse import mybir
from concourse.bass_utils import run_bass_kernel_spmd

F32 = mybir.dt.float32
F16 = mybir.dt.float16
I16 = mybir.dt.int16
I32 = mybir.dt.int32
AF = mybir.ActivationFunctionType
OP = mybir.AluOpType


@dataclass
class Cfg:
    N: int = 100000
    E: int = 3200000
    D_IN: int = 83
    D_HID: int = 1024
    NCLS: int = 25
    NCORES: int = 8
    NCH: int = 4             # source chunks for L1 (int16 idx range)
    XP: int = 128            # x_full row, f16 (256 B gather elem)
    PP: int = 32             # P row payload, f16
    GBLK: int = 8            # blocks per dma_gather call (1024 idx HW limit)
    IDXG: int = 8            # calls per idx staging DMA
    RING: int = 65536        # dynamic dma scratch (bytes) -> RING/16 descs
    NQ: int = 4              # SWDGE queues (gathers round-robin)
    WGRP: int = 4            # windows per L1 PSUM group (own bank each)

    @property
    def NSH(self):
        return self.N // self.NCORES

    @property
    def CH(self):
        return self.N // self.NCH

    @property
    def NW(self):            # L1 dest windows per core
        return (self.NSH + 127) // 128

    @property
    def NWD(self):           # L2 global dest windows
        return (self.N + 127) // 128

    @property
    def NJ(self):
        return self.D_HID // 128

    @property
    def NGRP(self):
        return (self.NW + self.WGRP - 1) // self.WGRP


def _wrap_idx16(idx: np.ndarray) -> np.ndarray:
    assert idx.size % 16 == 0
    a = idx.reshape(-1, 16).T.astype(np.int16)
    return np.tile(a, (8, 1))


# ----------------------------------------------------------------------------
# Host-side preprocessing
# ----------------------------------------------------------------------------

def _pack_blocks(order_keys, nblk_per_seg, seg_edges, gblk, chunk_of_seg):
    """Build the uniform block list + call list.

    order_keys: list of segment ids in emission order.
    nblk_per_seg: dict seg -> uniform block count.
    seg_edges: unused here (per-core packing is separate).
    Returns blocks [(seg, m, is_first_of_seg, is_last_of_seg)] and calls
    [(chunk, blk_start, nblk, off16)], where calls never span a chunk change.
    """
    blocks = []
    for seg in order_keys:
        mb = nblk_per_seg[seg]
        for m in range(mb):
            blocks.append((seg, m))
    calls = []
    i = 0
    while i < len(blocks):
        ch = chunk_of_seg(blocks[i][0])
        n = 1
        while (n < gblk and i + n < len(blocks)
               and chunk_of_seg(blocks[i + n][0]) == ch):
            n += 1
        calls.append([ch, i, n, 0])
        i += n
    off = 0
    for c in calls:
        c[3] = off
        off += c[2] * 128 // 16
    return blocks, calls, off


def build_host(cfg: Cfg, inputs: dict):
    N, NSH, NW, NCH, CH = cfg.N, cfg.NSH, cfg.NW, cfg.NCH, cfg.CH
    NWD = cfg.NWD

    feature = np.asarray(inputs["feature"], np.float32)
    conv_w = np.asarray(inputs["conv_w"], np.float32)
    conv_b = np.asarray(inputs["conv_b"], np.float32)
    W1 = np.asarray(inputs["W1"], np.float32)
    b1 = np.asarray(inputs["b1"], np.float32)
    W2 = np.asarray(inputs["W2"], np.float32)
    b2 = np.asarray(inputs["b2"], np.float32)
    val = np.asarray(inputs["adj_val"], np.float32)
    row = np.asarray(inputs["edge_row"], np.int64)
    col = np.asarray(inputs["edge_col"], np.int64)

    # conv1d(1->4, k=5, pad 2) summed over channels == banded matmul
    ws = conv_w.sum(axis=0).ravel()
    b0 = float(conv_b.sum())
    C = np.zeros((cfg.D_IN, 128), np.float32)
    for i in range(cfg.D_IN):
        for k in range(5):
            j = i - (k - 2)
            if 0 <= j < cfg.D_IN:
                C[i, j] = ws[k]

    ntile = (N + 127) // 128
    featT = np.zeros((cfg.D_IN, ntile * 128), np.float16)
    featT[:, :N] = feature.T.astype(np.float16)

    # ---------------- L1: edges by dest core, (window, chunk) segments ------
    core1 = row // NSH
    segs1 = [(w, ch) for g in range(cfg.NGRP)
             for ch in range(NCH)
             for w in range(g * cfg.WGRP, min((g + 1) * cfg.WGRP, NW))]
    per_core1 = []
    cnt1 = np.zeros((cfg.NCORES, NW, NCH), np.int64)
    for k in range(cfg.NCORES):
        m = core1 == k
        r, c_, v = row[m] - k * NSH, col[m], val[m]
        w = r >> 7
        ch = c_ // CH
        order = np.lexsort((c_, ch, w))
        r, c_, v, w, ch = r[order], c_[order], v[order], w[order], ch[order]
        key = w * NCH + ch
        cnt1[k] = np.bincount(key, minlength=NW * NCH).reshape(NW, NCH)
        per_core1.append((r, c_, v, key))
    M1 = np.ceil(cnt1.max(axis=0) / 128).astype(np.int64)   # [NW, NCH]
    M1[:, 0] = np.maximum(M1[:, 0], 1)  # every window needs >=1 block (phase C)

    nblk_of1 = {s: int(M1[s[0], s[1]]) for s in segs1}
    blocks1, calls1, tot16_1 = _pack_blocks(
        segs1, nblk_of1, None, cfg.GBLK, lambda s: s[1])
    nblk1 = len(blocks1)
    # first/last emitted block index per window (for PSUM start/stop), and
    # per half-group-of-4-windows (PSUM bank tile allocation)
    first1 = {}
    last1 = {}
    hg_first = {}
    for bi, (seg, m) in enumerate(blocks1):
        w = seg[0]
        if w not in first1:
            first1[w] = bi
        last1[w] = bi
        hg = w // 4
        if hg not in hg_first:
            hg_first[hg] = bi

    # ---------------- L2: edges by source core, global dest windows ---------
    core2 = col // NSH
    per_core2 = []
    cnt2 = np.zeros((cfg.NCORES, NWD), np.int64)
    for k in range(cfg.NCORES):
        m = core2 == k
        r2, c2, v2 = row[m], col[m] - k * NSH, val[m]
        wd = r2 >> 7
        order = np.lexsort((c2, wd))
        r2, c2, v2, wd = r2[order], c2[order], v2[order], wd[order]
        cnt2[k] = np.bincount(wd, minlength=NWD)
        per_core2.append((r2, c2, v2, wd))
    M2 = np.ceil(cnt2.max(axis=0) / 128).astype(np.int64)   # [NWD]
    M2 = np.maximum(M2, 1)  # ensure every window is written

    segs2 = list(range(NWD))
    nblk_of2 = {s: int(M2[s]) for s in segs2}
    blocks2, calls2, tot16_2 = _pack_blocks(
        segs2, nblk_of2, None, cfg.GBLK, lambda s: 0)
    nblk2 = len(blocks2)
    first2 = {}
    last2 = {}
    for bi, (seg, m) in enumerate(blocks2):
        if seg not in first2:
            first2[seg] = bi
        last2[seg] = bi

    # ---------------- per-core padded arrays --------------------------------
    in_maps = []
    W2p = np.zeros((cfg.D_HID, cfg.PP), np.float32)
    W2p[:, :cfg.NCLS] = W2
    b1c = b1.reshape(cfg.NJ, 128).T.copy()
    b2t = np.zeros((128, cfg.PP), np.float16)
    b2t[:, :cfg.NCLS] = b2[None, :]

    for k in range(cfg.NCORES):
        # L1 arrays
        r, c_, v, key = per_core1[k]
        pos = np.searchsorted(key, np.arange(NW * NCH + 1), side="left")
        idx1 = np.zeros(nblk1 * 128, np.int16)
        slot1 = np.zeros(nblk1 * 128, np.float32)
        val1 = np.zeros(nblk1 * 128, np.float32)
        blk_base = {}
        bi = 0
        for seg, m in blocks1:
            if m == 0:
                blk_base[seg] = bi
            bi += 1
        for (w, ch) in segs1:
            a, b = pos[w * NCH + ch], pos[w * NCH + ch + 1]
            n = b - a
            dst = blk_base[(w, ch)] * 128
            idx1[dst:dst + n] = (c_[a:b] % CH).astype(np.int16)
            slot1[dst:dst + n] = (r[a:b] - (w << 7)).astype(np.float32)
            val1[dst:dst + n] = v[a:b].astype(np.float32)
        idx1_arr = np.zeros((128, tot16_1), np.int16)
        for ch, bs, nb, o16 in calls1:
            seg = idx1[bs * 128:(bs + nb) * 128]
            idx1_arr[:, o16:o16 + nb * 8] = _wrap_idx16(seg)
        slot1_arr = slot1.reshape(nblk1, 128).T.copy()
        val1_arr = val1.reshape(nblk1, 128).T.copy()

        # L2 arrays
        r2, c2, v2, wd = per_core2[k]
        pos2 = np.searchsorted(wd, np.arange(NWD + 1), side="left")
        idx2 = np.zeros(nblk2 * 128, np.int16)
        slot2 = np.zeros(nblk2 * 128, np.float32)
        val2 = np.zeros(nblk2 * 128, np.float32)
        blk_base2 = {}
        bi = 0
        for seg, m in blocks2:
            if m == 0:
                blk_base2[seg] = bi
            bi += 1
        for s in segs2:
            a, b = pos2[s], pos2[s + 1]
            n = b - a
            dst = blk_base2[s] * 128
            idx2[dst:dst + n] = c2[a:b].astype(np.int16)
            slot2[dst:dst + n] = (r2[a:b] - (s << 7)).astype(np.float32)
            val2[dst:dst + n] = v2[a:b].astype(np.float32)
        idx2_arr = np.zeros((128, tot16_2), np.int16)
        for ch, bs, nb, o16 in calls2:
            seg = idx2[bs * 128:(bs + nb) * 128]
            idx2_arr[:, o16:o16 + nb * 8] = _wrap_idx16(seg)
        slot2_arr = slot2.reshape(nblk2, 128).T.copy()
        val2_arr = val2.reshape(nblk2, 128).T.copy()

        in_maps.append({
            "featT": featT,
            "Cmat": C.astype(np.float16),
            "W1": W1.astype(np.float16),
            "b1c": b1c,
            "W2p": W2p.astype(np.float16),
            "b2t": b2t,
            "idx1_dr": idx1_arr,
            "slot1_dr": slot1_arr,
            "val1_dr": val1_arr,
            "idx2_dr": idx2_arr,
            "slot2_dr": slot2_arr,
            "val2_dr": val2_arr,
        })

    meta = {
        "b0": b0, "ntile": ntile,
        "blocks1": blocks1, "calls1": calls1, "nblk1": nblk1,
        "tot16_1": tot16_1, "first1": first1, "last1": last1,
        "hg_first": hg_first,
        "blocks2": blocks2, "calls2": calls2, "nblk2": nblk2,
        "tot16_2": tot16_2, "first2": first2, "last2": last2,
        "empty2": {s for s in segs2 if cnt2[:, s].max() == 0},
    }
    return in_maps, meta


# ----------------------------------------------------------------------------
# Bass program
# ----------------------------------------------------------------------------

def build_program(cfg: Cfg, meta: dict) -> bass.Bass:
    NSH, NW, NCH, CH = cfg.NSH, cfg.NW, cfg.NCH, cfg.CH
    NWD, NJ, XP, PP = cfg.NWD, cfg.NJ, cfg.XP, cfg.PP
    ntile = meta["ntile"]
    blocks1, calls1 = meta["blocks1"], meta["calls1"]
    blocks2, calls2 = meta["blocks2"], meta["calls2"]
    nblk1, nblk2 = meta["nblk1"], meta["nblk2"]
    first1, last1 = meta["first1"], meta["last1"]
    first2, last2 = meta["first2"], meta["last2"]
    groups = [list(range(cfg.NCORES))]

    nc = bacc.Bacc("TRN2", target_bir_lowering=False, debug=False,
                   num_devices=cfg.NCORES,
                   dynamic_dma_scratch_size=cfg.RING,
                   num_swdge_queues=cfg.NQ)

    featT = nc.declare_dram_parameter("featT", [cfg.D_IN, ntile * 128], F16,
                                      isOutput=False)
    Cmat = nc.declare_dram_parameter("Cmat", [cfg.D_IN, 128], F16, isOutput=False)
    W1 = nc.declare_dram_parameter("W1", [cfg.D_IN, cfg.D_HID], F16, isOutput=False)
    b1c = nc.declare_dram_parameter("b1c", [128, NJ], F32, isOutput=False)
    W2p = nc.declare_dram_parameter("W2p", [cfg.D_HID, PP], F16, isOutput=False)
    b2t = nc.declare_dram_parameter("b2t", [128, PP], F16, isOutput=False)
    idx1_dr = nc.declare_dram_parameter("idx1_dr", [128, meta["tot16_1"]], I16,
                                        isOutput=False)
    slot1_dr = nc.declare_dram_parameter("slot1_dr", [128, nblk1], F32, isOutput=False)
    val1_dr = nc.declare_dram_parameter("val1_dr", [128, nblk1], F32, isOutput=False)
    idx2_dr = nc.declare_dram_parameter("idx2_dr", [128, meta["tot16_2"]], I16,
                                        isOutput=False)
    slot2_dr = nc.declare_dram_parameter("slot2_dr", [128, nblk2], F32, isOutput=False)
    val2_dr = nc.declare_dram_parameter("val2_dr", [128, nblk2], F32, isOutput=False)
    logits = nc.declare_dram_parameter("logits", [NSH, cfg.NCLS], F32, isOutput=True)

    x_full = nc.dram_tensor("x_full", [ntile * 128, XP], F16)
    p_sh = nc.dram_tensor("p_sh", [NSH, XP], F16)
    y_part = nc.dram_tensor("y_part", [cfg.N, PP], F16)
    y_red = nc.dram_tensor("y_red", [NSH, PP], F16)

    with tile.TileContext(nc) as tc:
        with (
            tc.tile_pool(name="singles", bufs=1) as singles,
            tc.tile_pool(name="feat", bufs=3) as featp,
            tc.tile_pool(name="work", bufs=4) as work,
            tc.tile_pool(name="sel", bufs=12) as selp,
            tc.tile_pool(name="gts", bufs=12) as gtsp,
            tc.tile_pool(name="gath", bufs=4) as gathp,
            tc.tile_pool(name="ht", bufs=18) as htp,
            tc.tile_pool(name="psg", bufs=1, space="PSUM") as psg,
            tc.tile_pool(name="psh", bufs=2, space="PSUM") as pshp,
            tc.tile_pool(name="psp", bufs=2, space="PSUM") as pspp,
        ):
            # ---------------- constants ----------------
            C_sb = singles.tile([cfg.D_IN, 128], F16)
            nc.sync.dma_start(out=C_sb[:], in_=Cmat[:])
            W1_sb = singles.tile([cfg.D_IN, cfg.D_HID], F16)
            nc.sync.dma_start(out=W1_sb[:], in_=W1[:])
            b1_sb = singles.tile([128, NJ], F32)
            nc.sync.dma_start(out=b1_sb[:], in_=b1c[:])
            W2_sb = singles.tile([128, NJ, PP], F16)
            nc.sync.dma_start(out=W2_sb[:], in_=W2p.rearrange("(j p) q -> p j q", p=128))
            b2_sb = singles.tile([128, PP], F16)
            nc.sync.dma_start(out=b2_sb[:], in_=b2t[:])
            slot1_sb = singles.tile([128, nblk1], F32)
            nc.sync.dma_start(out=slot1_sb[:], in_=slot1_dr[:])
            val1_sb = singles.tile([128, nblk1], F32)
            nc.sync.dma_start(out=val1_sb[:], in_=val1_dr[:])
            slot2_sb = singles.tile([128, nblk2], F32)
            nc.sync.dma_start(out=slot2_sb[:], in_=slot2_dr[:])
            val2_sb = singles.tile([128, nblk2], F32)
            nc.sync.dma_start(out=val2_sb[:], in_=val2_dr[:])

            b0_sb = singles.tile([128, 1], F32)
            nc.vector.memset(b0_sb[:], meta["b0"])
            iota_i = singles.tile([128, 128], I32)
            nc.gpsimd.iota(iota_i[:], pattern=[[1, 128]], base=0, channel_multiplier=0)
            iota_h = singles.tile([128, 128], F16)
            nc.vector.tensor_copy(out=iota_h[:], in_=iota_i[:])
            zero_y = singles.tile([128, PP], F16)
            nc.vector.memset(zero_y[:], 0.0)

            # ---------------- phase A: replicated conv ----------------
            FB = 8  # node-tiles per feature load / x_full write batch
            for t0 in range(0, ntile, FB):
                nb = min(FB, ntile - t0)
                ft = featp.tile([cfg.D_IN, FB * 128], F16, tag="ft")
                nc.sync.dma_start(out=ft[:, :nb * 128],
                                  in_=featT[:, t0 * 128:(t0 + nb) * 128])
                xt8 = work.tile([128, FB, 128], F16, tag="xt8", bufs=3)
                for j in range(nb):
                    t = t0 + j
                    ps_x = psg.tile([128, 128], F32, tag=f"gw{t % 4}",
                                    name=f"psxa{t % 4}")
                    nc.tensor.matmul(out=ps_x[:],
                                     lhsT=ft[:, j * 128:(j + 1) * 128],
                                     rhs=C_sb[:], start=True, stop=True)
                    nc.scalar.activation(out=xt8[:, j, :], in_=ps_x[:],
                                         func=AF.Relu, bias=b0_sb[:])
                nc.sync.dma_start(
                    out=x_full[t0 * 128:(t0 + nb) * 128, :cfg.D_IN].rearrange(
                        "(j p) c -> p j c", p=128),
                    in_=xt8[:, :nb, :cfg.D_IN])
            tc.strict_bb_all_engine_barrier()

            # ---------------- phase B+C: L1 SpMM + dense, per group --------
            idx_t = None
            g0 = 0
            ps_of_w = {}
            for ci, (ch, bs, nb, o16) in enumerate(calls1):
                if ci % cfg.IDXG == 0:
                    grp = calls1[ci:ci + cfg.IDXG]
                    g0 = o16
                    gn = sum(c[2] for c in grp) * 8
                    idx_t = work.tile([128, cfg.GBLK * 8 * cfg.IDXG], I16, tag="idx")
                    nc.sync.dma_start(out=idx_t[:, :gn], in_=idx1_dr[:, g0:g0 + gn])
                gt = gathp.tile([128, cfg.GBLK, XP], F16, tag="g1")
                nc.gpsimd.dma_gather(
                    out_ap=gt[:, :nb, :], in_ap=x_full[ch * CH:(ch + 1) * CH, :],
                    idxs_ap=idx_t[:, o16 - g0:o16 - g0 + nb * 8], num_idxs=nb * 128,
                    num_idxs_reg=nb * 128, elem_size=XP, queue_num=ci % cfg.NQ)
                for j in range(nb):
                    B = bs + j
                    (w, _ch), _m = blocks1[B]
                    wsize = min(128, NSH - w * 128)
                    sel = selp.tile([128, 128], F16, tag="sel")
                    nc.vector.tensor_scalar(
                        out=sel[:], in0=iota_h[:], scalar1=slot1_sb[:, B:B + 1],
                        scalar2=None, op0=OP.is_equal)
                    gts = gtsp.tile([128, 128], F16, tag="gts")
                    nc.scalar.activation(out=gts[:, :cfg.D_IN],
                                         in_=gt[:, j, :cfg.D_IN], func=AF.Copy,
                                         scale=val1_sb[:, B:B + 1])
                    if B == first1[w]:
                        ps_of_w[w] = psg.tile([128, 128], F32, tag=f"gw{w % 4}",
                                              name=f"psw{w % 4}")
                    pw = ps_of_w[w]
                    nc.tensor.matmul(out=pw[:cfg.D_IN, :],
                                     lhsT=gts[:, :cfg.D_IN], rhs=sel[:],
                                     start=(B == first1[w]), stop=(B == last1[w]))
                    if B == last1[w]:
                        # ---------------- phase C for window w ----------------
                        S1b = work.tile([cfg.D_IN, 128], F16, tag="s1b")
                        nc.scalar.activation(out=S1b[:], in_=pw[:cfg.D_IN, :],
                                             func=AF.Copy)
                        del ps_of_w[w]
                        hts = []
                        for jj in range(NJ):
                            ps_h = pshp.tile([128, 128], F32, tag="ph")
                            nc.tensor.matmul(
                                out=ps_h[:, :wsize],
                                lhsT=W1_sb[:, jj * 128:(jj + 1) * 128],
                                rhs=S1b[:, :wsize], start=True, stop=True)
                            ht = htp.tile([128, 128], F16, tag="ht")
                            nc.scalar.activation(
                                out=ht[:, :wsize], in_=ps_h[:, :wsize],
                                func=AF.Relu, bias=b1_sb[:, jj:jj + 1])
                            hts.append(ht)
                        ps_p = pspp.tile([128, PP], F32, tag="pp")
                        for jj in range(NJ):
                            nc.tensor.matmul(out=ps_p[:wsize], lhsT=hts[jj][:, :wsize],
                                             rhs=W2_sb[:, jj, :],
                                             start=(jj == 0), stop=(jj == NJ - 1))
                        pt = work.tile([128, PP], F16, tag="pt")
                        nc.scalar.activation(out=pt[:wsize], in_=ps_p[:wsize],
                                             func=AF.Copy)
                        nc.sync.dma_start(out=p_sh[w * 128:w * 128 + wsize, :PP],
                                          in_=pt[:wsize])

            # ---------------- phase D: L2 SpMM (source-partitioned) --------
            tc.strict_bb_all_engine_barrier()
            idx_t = None
            g0 = 0
            ps_d = None
            for ci, (ch, bs, nb, o16) in enumerate(calls2):
                if ci % cfg.IDXG == 0:
                    grp = calls2[ci:ci + cfg.IDXG]
                    g0 = o16
                    gn = sum(c[2] for c in grp) * 8
                    idx_t = work.tile([128, cfg.GBLK * 8 * cfg.IDXG], I16, tag="idx")
                    nc.sync.dma_start(out=idx_t[:, :gn], in_=idx2_dr[:, g0:g0 + gn])
                gt = gathp.tile([128, cfg.GBLK, XP], F16, tag="g1")
                nc.gpsimd.dma_gather(
                    out_ap=gt[:, :nb, :], in_ap=p_sh[:],
                    idxs_ap=idx_t[:, o16 - g0:o16 - g0 + nb * 8], num_idxs=nb * 128,
                    num_idxs_reg=nb * 128, elem_size=XP, queue_num=ci % cfg.NQ)
                for j in range(nb):
                    B = bs + j
                    wd, _m = blocks2[B]
                    wsize = min(128, cfg.N - wd * 128)
                    sel = selp.tile([128, 128], F16, tag="sel")
                    nc.vector.tensor_scalar(
                        out=sel[:], in0=iota_h[:], scalar1=slot2_sb[:, B:B + 1],
                        scalar2=None, op0=OP.is_equal)
                    gts = gtsp.tile([128, 128], F16, tag="gts")
                    nc.scalar.activation(out=gts[:, :PP], in_=gt[:, j, :PP],
                                         func=AF.Copy,
                                         scale=val2_sb[:, B:B + 1])
                    if B == first2[wd]:
                        ps_d = pspp.tile([128, PP], F32, tag="pp")
                    nc.tensor.matmul(out=ps_d[:wsize], lhsT=sel[:, :wsize],
                                     rhs=gts[:, :PP],
                                     start=(B == first2[wd]), stop=(B == last2[wd]))
                    if B == last2[wd]:
                        if wd in meta["empty2"]:
                            nc.sync.dma_start(
                                out=y_part[wd * 128:wd * 128 + wsize, :],
                                in_=zero_y[:wsize])
                        else:
                            yt = work.tile([128, PP], F16, tag="yt")
                            nc.scalar.activation(out=yt[:wsize], in_=ps_d[:wsize],
                                                 func=AF.Copy)
                            nc.sync.dma_start(
                                out=y_part[wd * 128:wd * 128 + wsize, :],
                                in_=yt[:wsize])

            # ---------------- ReduceScatter + bias ----------------
            tc.strict_bb_all_engine_barrier()
            nc.gpsimd.collective_compute(
                "ReduceScatter", OP.add, replica_groups=groups,
                ins=[y_part[:]], outs=[y_red[:]])
            tc.strict_bb_all_engine_barrier()
            for w in range(NW):
                wsize = min(128, NSH - w * 128)
                yr = work.tile([128, PP], F16, tag="yr")
                nc.sync.dma_start(out=yr[:wsize], in_=y_red[w * 128:w * 128 + wsize, :])
                lt = work.tile([128, PP], F32, tag="lt")
                nc.vector.tensor_add(out=lt[:wsize], in0=yr[:wsize],
                                     in1=b2_sb[:wsize])
                nc.sync.dma_start(out=logits[w * 128:w * 128 + wsize, :],
                                  in_=lt[:wsize, :cfg.NCLS])

    nc.compile()
    return nc


# ----------------------------------------------------------------------------
# Entry point
# ----------------------------------------------------------------------------

def _run(cfg: Cfg, inputs: dict, trace: bool = False):
    in_maps, meta = build_host(cfg, inputs)
    nc = build_program(cfg, meta)
    res = run_bass_kernel_spmd(nc, in_maps, list(range(cfg.NCORES)), trace=trace)
    out = np.concatenate([res.results[k]["logits"] for k in range(cfg.NCORES)], axis=0)
    return out, res


def kernel(**inputs) -> np.ndarray:
    cfg = Cfg()
    out, _ = _run(cfg, inputs, trace=False)
    return out.astype(np.float32)


if __name__ == "__main__":
    # smoke test at reduced scale against a numpy reference
    cfg = Cfg(N=2048, E=32768, NCH=2)
    rng = np.random.default_rng(0)
    inputs = {
        "feature": rng.standard_normal((cfg.N, cfg.D_IN), dtype=np.float32),
        "conv_w": rng.standard_normal((4, 1, 5), dtype=np.float32) * 0.2,
        "conv_b": np.zeros(4, np.float32),
        "W1": rng.standard_normal((cfg.D_IN, cfg.D_HID), dtype=np.float32) * 0.1,
        "b1": np.zeros(cfg.D_HID, np.float32),
        "W2": rng.standard_normal((cfg.D_HID, cfg.NCLS), dtype=np.float32) * 0.05,
        "b2": np.zeros(cfg.NCLS, np.float32),
        "adj_val": rng.random(cfg.E, dtype=np.float32),
        "edge_row": rng.integers(0, cfg.N, cfg.E).astype(np.int32),
        "edge_col": rng.integers(0, cfg.N, cfg.E).astype(np.int32),
    }
    out, _ = _run(cfg, inputs)

    ws = inputs["conv_w"].sum(axis=0).ravel()
    xr = np.zeros((cfg.N, cfg.D_IN), np.float32)
    f = inputs["feature"]
    for k in range(5):
        s = k - 2
        lo, hi = max(0, -s), min(cfg.D_IN, cfg.D_IN - s)
        xr[:, lo:hi] += ws[k] * f[:, lo + s:hi + s]
    xr = np.maximum(xr + inputs["conv_b"].sum(), 0)
    S1 = np.zeros_like(xr)
    np.add.at(S1, inputs["edge_row"],
              inputs["adj_val"][:, None] * xr[inputs["edge_col"]])
    h = np.maximum(S1 @ inputs["W1"] + inputs["b1"], 0)
    P = h @ inputs["W2"]
    Y = np.zeros_like(P)
    np.add.at(Y, inputs["edge_row"], inputs["adj_val"][:, None] * P[inputs["edge_col"]])
    Y += inputs["b2"]
    err = np.abs(out - Y).max() / (np.abs(Y).max() + 1e-30)
    print("rel err:", err)
